# revision 26
# baseline (speedup 1.0000x reference)
"""Trainium2 Bass kernel for fused embedding-lookup -> mean-pool -> dot(weights).

Reference computation (B=16384, L=200, D=100, V=100000):
    out[b] = mean_l(embed_table[word_idxs[b, l], :]) @ weights            # [B, 1]

Key algebraic transform: the dot with `weights` is linear, so
    out[b] = sum_l s[word_idxs[b, l]],   with  s = embed_table @ (weights / L)
Instead of gathering B*L rows of 400B (1.31 GB), we precompute the V-element
vector `s` on-device (the 40MB table is read exactly once across the 8 cores)
and gather B*L scalars.

The scalar gather uses the TIE-ucode `dma_gather` (int16 row indices). Its
row stride must be a 256B multiple, so s is spread into a bf16 table
    S16[j, 0:16] = s_pad[16j .. 16j+16),  row pitch 128 bf16 = 256B,
and a token with index v gathers row j = v >> 4 (32B payload), after which
a 16-wide mask+reduce selects lane r = v & 15 and accumulates per
row-group. (16 values per row keeps the table write at 1.6MB / ~4.5us on
the startup critical path; the wider DVE select hides under the gather.)

Performance structure (HW-measured): the wall is Q7 descriptor GENERATION
inside the dma_gather ucode — each gather runs on one Q7 core pair (pair =
queue_num) at ~9.4 cycles/descriptor, one descriptor per token. All four
queue pairs generate concurrently (the 8 Q7 cores pop the NX instruction
queue asynchronously), so gathers are issued as four per-queue chains of
SMALL (2-block) instructions with phase-shifted first sizes: small
instructions keep >1 instruction resident per descriptor ring, so descgen
runs back-to-back instead of stalling on the NX-decode ring-space await
(which also head-of-line blocks later queues). Per-core: 409600 tokens at
~2.0ns/token effective -> ~0.83ms gather phase.

Sharding (8 cores): batch-parallel gather (2048 rows/core); vocab-parallel s
precompute in bf16 (12800 padded rows/core) + bf16 AllGather, then a
one-shot SBUF-built spread + a DRAM write split across the two HWDGE
queues (never per-row descriptors).

Startup engineering (first gather at ~86us, was ~98us): the mlp gather-
ucode library is loaded at t=0 (hides the ~10us auto-inserted LOAD_LIB);
the table load is split across the Sync+Scalar HWDGE queues and ordered
ahead of the prefetches (s -> AllGather -> spread is the critical chain);
the DVE s-reduction is split (bf16 halves add at 2x rate + X-reduce over
D/2); mask-side inputs load on the Scalar queue; the last wave's masks are
hoisted into the ~40us collective wait (they otherwise serialize into the
post-gather tail).  The ~40us AllGather (~16us CC-stream launch latency +
ring hops) is the irreducible startup term; overlapping it with gathers of
local-vocab tokens was tried and measured SLOWER (requires shrinking the
gather blocks to 12 slots, which costs ~1.7ns/desc in descgen efficiency —
~150us total; see git history).

Host does layout only: shard/reshape inputs, compute j = idx>>4 / r = idx&15,
wrap indices in the dma_gather [16, S] layout, and concat per-core outputs.
"""

import os
import sys

import numpy as np

for _p in ("/opt/trn_rl_repo",):
    if os.path.isdir(_p) and _p not in sys.path:
        sys.path.insert(0, _p)

from concourse import bacc, bass, library_config, mybir, tile  # noqa: E402
from concourse.bass_utils import run_bass_kernel_spmd  # noqa: E402

F32 = mybir.dt.float32
BF16 = mybir.dt.bfloat16
I32 = mybir.dt.int32
I16 = mybir.dt.int16
P = 128
NCORES = 8


def dma_gather_raw(
    gp, out_ap, in_ap, idxs_ap, num_idxs, num_idxs_reg, elem_size, elem_step,
    queue_num=0, single_packet=False,
):
    """nc.gpsimd.dma_gather minus the 256B *element* restriction.

    Only the source row PITCH must be a 256B multiple (stride_bytes_256 is an
    8-bit field in 256B units); the per-index element payload can be smaller.
    Emits the same InstDMAGatherAnt the stock wrapper does.
    """
    dt_sz = mybir.dt.size(in_ap.dtype)
    stride_256 = (elem_step * dt_sz) // 256
    assert elem_step * dt_sz == stride_256 * 256 and 0 < stride_256 < 256
    assert in_ap.ap[0][0] == elem_step and in_ap.ap[-1][1] == elem_size
    _in_ap = gp.lower_ap_dma(in_ap, for_custom_bir_dma=True)
    _idxs_ap = gp.lower_ap(idxs_ap)
    _out_ap = gp.lower_ap(out_ap)
    return gp.add_instruction(
        mybir.InstDMAGatherAnt(
            name=gp.bass.get_next_instruction_name(),
            ins=[*_in_ap, _idxs_ap, gp.lower_val_access(gp.to_reg(num_idxs_reg))],
            outs=[_out_ap],
            transpose=False,
            num_idxs=num_idxs,
            elem_size=elem_size,
            stride_bytes_256=stride_256,
            gen_mode=0,
            single_packet=single_packet,
            queue_num=queue_num,
            sbuf_tokens_per_rank=0,
            sbuf_free_dim_per_rank=0,
            sbuf_free_dim_pad_per_rank=0,
            sbuf_byte_offset=0,
        )
    )


def build_program(
    G=16, L=200, D=100, RPP=98, CPI=100, NQ=4, ncores=NCORES, use_collective=True,
    repeat=1, ELEM=16, GAT_BUFS=12, SINGLE_PACKET=False, quad_split=False,
):
    """Build the SPMD program (identical on all cores).

    G:   row-groups per core (batch rows per core = G*128)
    L:   tokens per row
    D:   embedding dim
    RPP: padded vocab rows per SBUF partition (vocab rows per core = 128*RPP)
    CPI: token-slot BLOCK size (gather instructions cover 1..4 blocks);
         L % CPI == 0
    NQ:  SWDGE queues to rotate over (1..4)

    Gather instructions are grouped into per-queue chains whose first (and
    last) instructions have staggered sizes (1/2/3/4 blocks). Same-queue
    instructions serialize through descriptor-ring space, so equal-sized
    chains would fall into lockstep: all four queues' SDMA drains collide
    after each descgen wave and every queue idles through the combined
    drain. Staggered chain heads phase-shift the queues so each queue's
    drain overlaps the other queues' descgen.
    """
    assert L % CPI == 0
    BLK = CPI  # token slots per block
    SLOTS = G * L  # token slots per partition
    NBLK = SLOTS // BLK  # total blocks
    HB = L // BLK  # blocks per row-group
    ICOL = P * BLK // 16  # idxw columns per block
    # Max blocks per gather instruction. 2 is measured-optimal: at 2 blocks
    # (400 descs/engine) each queue's descriptor ring holds ~2.5
    # instructions and descgen runs back-to-back; at 3 blocks (600/engine,
    # <2 resident) descgen-drain lockstep returns and costs ~350us; at 4
    # it costs ~70us. Do not raise without re-measuring.
    MAXB = 2
    # Per-queue chains in blocks. Small instructions keep each queue's
    # descriptor ring holding >1 instruction, so Q7 descgen runs
    # back-to-back instead of stalling on the NX-decode ring-space await
    # (which also head-of-line blocks later queues' decode). Varied first
    # sizes phase-shift the queues.
    #
    # Equal per-queue split. (The trace shows queue 0 running gapless at
    # ~7.9ns/desc while queues 1-3 idle between ~23us instruction windows —
    # but rebalancing blocks toward queues 1-3 (17/37/37/37) measured
    # 1.26ms vs 0.94ms: the apparent fast-pair headroom is a dispatch
    # artifact, not spare descgen capacity. Keep the split equal.)
    if quad_split:
        # Phase-L: blocks {8g, 8g+1} of each row hold QUAD-local tokens
        # (vocab in this core's 4-core group slice) and gather from the
        # quad slab, which is ready after a fast 4-core AllGather; the
        # 8-core AllGather then completes under ~94us of phase-L gather
        # work. Global block numbering is unchanged so the reduce / idxw
        # / r2 slicing is identical. q0/q2 split their first L-pair into
        # 1-block instructions — the persistent half-instruction phase
        # offset that keeps the queues' SDMA drains de-phased.
        assert HB == 8 and MAXB == 2 and G % NQ == 0
        lch = {q: [] for q in range(NQ)}
        rch = {q: [] for q in range(NQ)}
        for g in range(G):
            q = g % NQ
            b0 = HB * g
            if g < NQ and q % 2 == 0:
                lch[q] += [(b0, 1), (b0 + 1, 1)]
            else:
                lch[q] += [(b0, 2)]
            rch[q] += [(b0 + 2, 2), (b0 + 4, 2), (b0 + 6, 2)]
        plan = []
        for part in (lch, rch):
            pos = {q: 0 for q in range(NQ)}
            while any(pos[q] < len(part[q]) for q in range(NQ)):
                for q in range(NQ):
                    if pos[q] < len(part[q]):
                        b0, n = part[q][pos[q]]
                        pos[q] += 1
                        plan.append((q, b0, n))
        NLINST = sum(len(v) for v in lch.values())
        assert sum(n for _, _, n in plan) == NBLK
    else:
        QBLOCKS = [NBLK // NQ] * NQ
        plan = []  # (queue, first block, nblocks), in issue order
        chains = []
        for q in range(NQ):
            first = (q % MAXB) + 1
            rest = QBLOCKS[q] - first
            sizes = [first] + [MAXB] * (rest // MAXB)
            if rest % MAXB:
                sizes.append(rest % MAXB)
            chains.append(sizes)
        blk0 = 0
        pos = [0] * NQ
        while any(pos[q] < len(chains[q]) for q in range(NQ)):
            for q in range(NQ):
                if pos[q] < len(chains[q]):
                    n = chains[q][pos[q]]
                    pos[q] += 1
                    plan.append((q, blk0, n))
                    blk0 += n
        assert blk0 == NBLK
        NLINST = 0
    VPC = P * RPP
    V_PAD = VPC * ncores
    # spread-table geometry: row j holds s[ELEM*j .. ELEM*j + ELEM) in the
    # first ELEM bf16 lanes of a 256B-pitch row. Wider rows shrink the
    # table (and its critical-path DRAM write) at the cost of a wider
    # DVE select, which is hidden under the gather waves.
    NROWS = V_PAD // ELEM
    assert NROWS % P == 0 and NROWS <= 32768
    nc = bacc.Bacc(
        "TRN2",
        target_bir_lowering=False,
        debug=False,
        num_devices=ncores,
        num_swdge_queues=NQ,
    )
    idxw_t = nc.dram_tensor("idxw", [P, SLOTS * 8], I16, kind="ExternalInput")
    io4_t = nc.dram_tensor("io4", [P, ELEM], BF16, kind="ExternalInput")
    r2_t = nc.dram_tensor("r2", [P, SLOTS], BF16, kind="ExternalInput")
    tab_t = nc.dram_tensor("tab", [P, RPP * D], BF16, kind="ExternalInput")
    w_t = nc.dram_tensor("w", [P, D], BF16, kind="ExternalInput")
    out_t = nc.dram_tensor("out", [P, G], F32, kind="ExternalOutput")

    with tile.TileContext(nc) as tc:
        with (
            tc.tile_pool(name="dr", bufs=1, space="DRAM") as dr,
            tc.tile_pool(name="keep", bufs=1) as keep,
        ):
            # Load the gather ucode library up front (gpsimd is idle during
            # startup); otherwise the auto-inserted LOAD_LIB lands right
            # before the first gather and costs ~10us on the critical path.
            nc.gpsimd.load_library(library_config.mlp)

            # ---- stage 1 loads first: the s -> collective -> spread chain
            # is the startup critical path, so the table halves go to the
            # two HWDGE queues (Sync + Scalar) before anything else.
            tab_sb = keep.tile([P, RPP * D], BF16, name="tab_sb")
            # two-way split across the HWDGE queues (a third slice on the
            # gpsimd SWDGE queue was tried and lands LATE — it delays the
            # s-compute ~9us)
            HALF = RPP * D // 2
            nc.sync.dma_start(tab_sb[:, 0:HALF], tab_t[:, 0:HALF])
            nc.scalar.dma_start(tab_sb[:, HALF:], tab_t[:, HALF:])
            w_sb = keep.tile([P, D], BF16, name="w_sb")
            nc.scalar.dma_start(w_sb[:], w_t[:])

            # ---- stage 0: prefetch gather-side inputs ----
            # idxw on Sync right behind the tab half (needed from ~90us);
            # mask-side inputs on Scalar.
            first_idxw = {}
            for q, b0, nb in plan[: NQ]:
                t_ = keep.tile([P, MAXB * ICOL], I16, name=f"idxwf{b0}")
                nc.sync.dma_start(
                    t_[:, 0 : nb * ICOL],
                    idxw_t[:, b0 * ICOL : (b0 + nb) * ICOL],
                )
                first_idxw[b0] = t_
            iota4 = keep.tile([P, ELEM], BF16)
            nc.scalar.dma_start(iota4[:], io4_t[:])
            r2_sb = keep.tile([P, SLOTS], BF16)
            nc.scalar.dma_start(r2_sb[:], r2_t[:])
            half_sb = keep.tile([P, NBLK], F32)
            out_sb = keep.tile([P, G], F32)

            with tc.tile_pool(name="pre", bufs=1) as pre:
                # ---- stage 1: s_part = (table slice) @ (w/L) ----
                # bf16 table/weights: halves the 5MB load and doubles DVE
                # throughput for the product; the reduce accumulates to f32.
                # Split the reduction: a bf16 tensor_tensor add of the two
                # D/2 halves (2x DVE rate) then an X-reduce over D/2 —
                # measured faster than one X-reduce over D (~10.6us -> ~7us).
                prod_sb = pre.tile([P, RPP * D], BF16)
                nc.vector.tensor_tensor(
                    out=prod_sb[:].rearrange("p (r d) -> p r d", d=D),
                    in0=tab_sb[:].rearrange("p (r d) -> p r d", d=D),
                    in1=w_sb[:].unsqueeze(1).to_broadcast([P, RPP, D]),
                    op=mybir.AluOpType.mult,
                )
                D2 = D // 2
                psum_sb = pre.tile([P, RPP * D2], BF16)
                nc.vector.tensor_tensor(
                    out=psum_sb[:].rearrange("p (r d) -> p r d", d=D2),
                    in0=prod_sb[:].rearrange("p (r d) -> p r d", d=D)[:, :, 0:D2],
                    in1=prod_sb[:].rearrange("p (r d) -> p r d", d=D)[:, :, D2:D],
                    op=mybir.AluOpType.add,
                )
                s_sb = pre.tile([P, RPP], F32)
                nc.vector.tensor_reduce(
                    out=s_sb[:].unsqueeze(2),
                    in_=psum_sb[:].rearrange("p (r d) -> p r d", d=D2),
                    axis=mybir.AxisListType.X,
                    op=mybir.AluOpType.add,
                )

                # ---- stage 2: AllGather s (bf16) ----
                # The SWDGE write casts f32 -> bf16, halving the collective
                # payload (200KB).
                s_part = dr.tile([P, RPP], BF16)
                nc.gpsimd.dma_start(s_part[:], s_sb[:])
                RPW = NROWS // P  # spread rows per partition
                S16 = dr.tile([NROWS, 128], BF16)
                NQUAD = 4
                if quad_split:
                    assert use_collective
                    # fast 4-core AllGather first; the 8-core one (variable
                    # 40-88us: it waits on the slowest core) follows on the
                    # CC stream and completes under the phase-L gathers.
                    # Shared output: the non-Shared (local) out tile takes
                    # the CC bounce path (~230us to data-ready, measured).
                    # NOTE this assumes Shared scratchpad buffers are
                    # per-core copies — if they were one physical buffer
                    # the two quad groups would clobber each other (the
                    # rel-err gate catches that).
                    sq_full = dr.tile(
                        [NQUAD * RPP, P], BF16, addr_space="Shared"
                    )
                    nc.gpsimd.collective_compute(
                        "AllGather",
                        mybir.AluOpType.bypass,
                        replica_groups=[[0, 1, 2, 3], [4, 5, 6, 7]],
                        ins=[s_part.opt()],
                        outs=[sq_full.opt()],
                    )
                if use_collective:
                    s_full = dr.tile([ncores * RPP, P], BF16, addr_space="Shared")
                    nc.gpsimd.collective_compute(
                        "AllGather",
                        mybir.AluOpType.bypass,
                        replica_groups=[list(range(ncores))],
                        ins=[s_part.opt()],
                        outs=[s_full.opt()],
                    )
                else:
                    # crash-isolation mode: fill s_full with the local part
                    # replicated (wrong data, same program shape)
                    s_full = dr.tile([ncores * RPP, P], BF16)
                    for c in range(ncores):
                        nc.sync.dma_start(
                            s_full[c * RPP : (c + 1) * RPP, :],
                            s_part[:].rearrange("p r -> (p r)").rearrange(
                                "(r q) -> r q", q=P
                            ),
                        )

                # ---- stage 3a: quad spread (phase-L gather table) ----
                if quad_split:
                    NROWSQ = NQUAD * VPC // ELEM
                    RPWQ = NROWSQ // P
                    S16Q = dr.tile([NROWSQ, 128], BF16)
                    sqf_sb = pre.tile([P, RPWQ * ELEM], BF16)
                    # scalar queue: free after its startup loads; sync
                    # still feeds phase-L idxw tiles.
                    nc.scalar.dma_start(
                        sqf_sb[:],
                        sq_full[:]
                        .rearrange("a b -> (a b)")
                        .rearrange("(p x) -> p x", p=P),
                    )
                    sspq_sb = pre.tile([P, RPWQ * 128], BF16)
                    nc.vector.tensor_copy(
                        out=sspq_sb[:].rearrange("p (r k) -> p r k", k=128)[
                            :, :, 0:ELEM
                        ],
                        in_=sqf_sb[:].rearrange("p (r q) -> p r q", q=ELEM),
                    )
                    S16Qflat = S16Q[:].rearrange("a b -> (a b)").rearrange(
                        "(p x) -> p x", p=P
                    )
                    nc.scalar.dma_start(S16Qflat[:], sspq_sb[:])
                else:
                    S16Q = S16

            # Hoist the LAST wave's masks: they depend only on r2/io4
            # (landed ~20us) and otherwise serialize into the post-gather
            # tail. Emitted here so the DVE computes them during the
            # collective wait.
            full_plan = plan * repeat
            hoisted = {}
            for q, b0, nb in full_plan[-NQ:]:
                n = nb * BLK
                m_ = keep.tile([P, MAXB * BLK, ELEM], BF16, name=f"maskh{b0}")
                nc.vector.tensor_tensor(
                    out=m_[:, 0:n, :],
                    in0=r2_sb[:, b0 * BLK : b0 * BLK + n]
                    .unsqueeze(2)
                    .to_broadcast([P, n, ELEM]),
                    in1=iota4[:].unsqueeze(1).to_broadcast([P, n, ELEM]),
                    op=mybir.AluOpType.is_equal,
                )
                hoisted[b0] = m_

            with tc.tile_pool(name="gat", bufs=GAT_BUFS) as gat:
                # ---- stage 4: gather + select + reduce ----
                def emit_gather(i, q, b0, nb):
                    n = nb * BLK
                    # phase-L blocks (first 2 of each row) read the quad
                    # slab; everything else reads the full table.
                    src = S16Q if (quad_split and b0 % HB < MAXB) else S16
                    if i < NQ:
                        idxw_sb = first_idxw[b0]
                    else:
                        idxw_sb = gat.tile(
                            [P, MAXB * ICOL], I16, tag="idxw", name=f"idxw{b0}"
                        )
                        nc.sync.dma_start(
                            idxw_sb[:, 0 : nb * ICOL],
                            idxw_t[:, b0 * ICOL : (b0 + nb) * ICOL],
                        )
                    gth = gat.tile(
                        [P, MAXB * BLK, ELEM], BF16, tag="gth", name=f"gth{b0}"
                    )
                    dma_gather_raw(
                        nc.gpsimd,
                        gth[:, 0:n, :],
                        src[:, 0:ELEM],
                        idxw_sb[:, 0 : nb * ICOL],
                        P * n,
                        P * n,
                        elem_size=ELEM,
                        elem_step=128,
                        queue_num=q,
                        single_packet=SINGLE_PACKET,
                    )
                    if b0 in hoisted:
                        mask = hoisted[b0]
                    else:
                        mask = gat.tile(
                            [P, MAXB * BLK, ELEM], BF16, tag="mask", name=f"mask{b0}"
                        )
                        nc.vector.tensor_tensor(
                            out=mask[:, 0:n, :],
                            in0=r2_sb[:, b0 * BLK : b0 * BLK + n]
                            .unsqueeze(2)
                            .to_broadcast([P, n, ELEM]),
                            in1=iota4[:].unsqueeze(1).to_broadcast([P, n, ELEM]),
                            op=mybir.AluOpType.is_equal,
                        )
                    msel = gat.tile(
                        [P, MAXB * BLK, ELEM], BF16, tag="msel", name=f"msel{b0}"
                    )
                    nc.vector.tensor_tensor(
                        out=msel[:, 0:n, :],
                        in0=mask[:, 0:n, :],
                        in1=gth[:, 0:n, 0:ELEM],
                        op=mybir.AluOpType.mult,
                    )
                    nc.vector.tensor_reduce(
                        out=half_sb[:, b0 : b0 + nb].unsqueeze(2),
                        in_=msel[:, 0:n, :]
                        .rearrange("p a b -> p (a b)")
                        .rearrange("p (n x) -> p n x", x=BLK * ELEM),
                        axis=mybir.AxisListType.X,
                        op=mybir.AluOpType.add,
                    )

                for i, (q, b0, nb) in enumerate(full_plan[:NLINST]):
                    emit_gather(i, q, b0, nb)

                # ---- stage 3b: full-table spread, emitted AFTER phase-L
                # so the in-order DVE queue never blocks phase-L's
                # mask/select work on the (variable-latency) 8-core
                # AllGather. For quad_split=False, NLINST=0 and this is
                # the original pre-gather position.
                sfull_sb = keep.tile([P, RPW * ELEM], BF16, name="sfull_sb")
                nc.sync.dma_start(
                    sfull_sb[:],
                    s_full[:]
                    .rearrange("a b -> (a b)")
                    .rearrange("(p x) -> p x", p=P),
                )
                ssp_sb = keep.tile([P, RPW * 128], BF16, name="ssp_sb")
                nc.vector.tensor_copy(
                    out=ssp_sb[:].rearrange("p (r k) -> p r k", k=128)[
                        :, :, 0:ELEM
                    ],
                    in_=sfull_sb[:].rearrange("p (r q) -> p r q", q=ELEM),
                )
                S16flat = S16[:].rearrange("a b -> (a b)").rearrange(
                    "(p x) -> p x", p=P
                )
                XH = RPW * 128 // 2
                nc.sync.dma_start(S16flat[:, 0:XH], ssp_sb[:, 0:XH])
                nc.scalar.dma_start(S16flat[:, XH:], ssp_sb[:, XH:])

                for j, (q, b0, nb) in enumerate(full_plan[NLINST:]):
                    emit_gather(NLINST + j, q, b0, nb)
                nc.vector.tensor_reduce(
                    out=out_sb[:].unsqueeze(2),
                    in_=half_sb[:].rearrange("p (g h) -> p g h", h=HB),
                    axis=mybir.AxisListType.X,
                    op=mybir.AluOpType.add,
                )
                nc.sync.dma_start(out_t[:], out_sb[:])
    nc.compile()
    return nc


def make_in_maps(
    word_idxs, embed_table, weights, G, L, D, RPP, CPI, ncores=NCORES,
    quad_split=False,
):
    """Shard + lay out the full inputs for the per-core program."""
    BPC = G * P
    SLOTS = G * L
    NT = SLOTS // CPI
    VPC = P * RPP
    import ml_dtypes

    bf16 = ml_dtypes.bfloat16
    idx = np.asarray(word_idxs).astype(np.int32)
    tab = np.asarray(embed_table, dtype=np.float32)
    w = np.asarray(weights, dtype=np.float32).reshape(-1)
    V = tab.shape[0]
    tab_pad = np.zeros((VPC * ncores, D), dtype=bf16)
    tab_pad[:V] = tab.astype(bf16)
    w_c = np.ascontiguousarray(
        np.broadcast_to((w / np.float32(L))[None, :].astype(bf16), (P, D))
    )
    in_maps = []
    for c in range(ncores):
        # token slot layout: [partition p, slot j=g*L+l] holds idx of batch
        # row (c*BPC + g*128 + p), token l
        rows = idx[c * BPC : (c + 1) * BPC]  # [BPC, L]
        if quad_split:
            # quad-local tokens first: the first 2*CPI slots of each row
            # are gathered from the quad slab (phase-L)
            Q = c // 4
            is_q = (rows // (4 * VPC)) == Q
            assert is_q.sum(axis=1).min() >= 2 * CPI
            order = np.argsort(~is_q, axis=1, kind="stable")
            rows = np.take_along_axis(rows, order, axis=1)
        slots = rows.reshape(G, P, L).transpose(1, 0, 2).reshape(P, SLOTS)
        jmat = (slots >> 4).astype(np.int16)  # [P, SLOTS]
        r2 = (slots & 15).astype(bf16)
        if quad_split:
            # phase-L slots index the quad slab (lane = v & 15 unchanged:
            # 4*VPC is a multiple of 16)
            QL = 2 * CPI
            sl3 = slots.reshape(P, G, L)
            j3 = jmat.reshape(P, G, L)
            j3[:, :, 0:QL] = (
                (sl3[:, :, 0:QL] - (c // 4) * 4 * VPC) >> 4
            ).astype(np.int16)
        # per-instruction index lists in i = c_local*128 + p order, wrapped
        # into the dma_gather [16, NI//16] layout, replicated to 128 parts
        u = jmat.reshape(P, NT, CPI).transpose(1, 2, 0)  # [NT, CPI, P]
        wrp = u.reshape(NT, CPI * P // 16, 16).transpose(2, 0, 1).reshape(16, -1)
        idxw = np.ascontiguousarray(np.tile(wrp, (8, 1)))  # [128, SLOTS*8]
        tab_c = np.ascontiguousarray(
            tab_pad[c * VPC : (c + 1) * VPC].reshape(P, RPP * D)
        )
        in_maps.append(
            {
                "idxw": idxw,
                "r2": np.ascontiguousarray(r2),
                "tab": tab_c,
                "w": w_c,
                "io4": np.ascontiguousarray(
                    np.broadcast_to(
                        np.arange(16, dtype=np.float32).astype(bf16), (P, 16)
                    )
                ),
            }
        )
    return in_maps


def unshard_out(results, G, ncores=NCORES):
    """results: list of per-core {'out': [128, G]} -> full [B, 1] f32."""
    parts = []
    for c in range(ncores):
        o = np.asarray(results[c]["out"])  # [P, G]; out[p, g] = row g*128+p
        parts.append(o.T.reshape(-1))
    return np.concatenate(parts).reshape(-1, 1).astype(np.float32)


_CACHED_NC = None

FULL = dict(G=16, L=200, D=100, RPP=100, CPI=25)


def _quad_ok(idx):
    """quad_split needs >= 2*CPI quad-local tokens in EVERY row."""
    BPC = FULL["G"] * P
    qvpc = 4 * P * FULL["RPP"]
    for c in range(NCORES):
        rows = idx[c * BPC : (c + 1) * BPC]
        if ((rows // qvpc) == (c // 4)).sum(axis=1).min() < 2 * FULL["CPI"]:
            return False
    return True


def _get_nc(quad_split):
    global _CACHED_NC
    if _CACHED_NC is None or _CACHED_NC[1] != quad_split:
        _CACHED_NC = (build_program(**FULL, quad_split=quad_split), quad_split)
    return _CACHED_NC[0]


def run(word_idxs, embed_table, weights, trace=False, **spmd_kwargs):
    """Build (cached), run on the 8 cores, return (full_out, BassKernelResults)."""
    idx = np.asarray(word_idxs).astype(np.int32)
    # quad_split is dead on this stack: a 4-core-group AllGather cannot
    # use the Shared-output fast path ("needs >4 cores") and the local-out
    # bounce path is ~230us to data-ready (measured 1175us total); an
    # 8-core prefix collective would pay the same ~40-88us latency as the
    # full one (latency, not payload, dominates). Keep False.
    quad_split = False and _quad_ok(idx)
    nc = _get_nc(quad_split)
    in_maps = make_in_maps(
        idx,
        embed_table,
        weights,
        FULL["G"],
        FULL["L"],
        FULL["D"],
        FULL["RPP"],
        FULL["CPI"],
        quad_split=quad_split,
    )
    res = run_bass_kernel_spmd(
        nc, in_maps, core_ids=list(range(NCORES)), trace=trace, **spmd_kwargs
    )
    out = unshard_out(res.results, FULL["G"])
    return out, res


def kernel(word_idxs, embed_table, weights):
    out, _ = run(word_idxs, embed_table, weights, trace=False)
    return out



# revision 28
# speedup vs baseline: 1.1812x; 1.1812x over previous
"""Trainium2 Bass kernel for fused embedding-lookup -> mean-pool -> dot(weights).

Reference computation (B=16384, L=200, D=100, V=100000):
    out[b] = mean_l(embed_table[word_idxs[b, l], :]) @ weights            # [B, 1]

Key algebraic transform: the dot with `weights` is linear, so
    out[b] = sum_l s[word_idxs[b, l]],   with  s = embed_table @ (weights / L)
Instead of gathering B*L rows of 400B (1.31 GB), we precompute the V-element
vector `s` on-device (the 40MB table is read exactly once across the 8 cores)
and gather B*L scalars.

The scalar gather uses the TIE-ucode `dma_gather` (int16 row indices). Its
row stride must be a 256B multiple, so s is spread into a bf16 table
    S16[j, 0:16] = s_pad[16j .. 16j+16),  row pitch 128 bf16 = 256B,
and a token with index v gathers row j = v >> 4 (32B payload), after which
a 16-wide mask+reduce selects lane r = v & 15 and accumulates per
row-group. (16 values per row keeps the table write at 1.6MB / ~4.5us on
the startup critical path; the wider DVE select hides under the gather.)

Performance structure (HW-measured): the wall is Q7 descriptor GENERATION
inside the dma_gather ucode — each gather runs on one Q7 core pair (pair =
queue_num) at ~9.4 cycles/descriptor, one descriptor per token. All four
queue pairs generate concurrently (the 8 Q7 cores pop the NX instruction
queue asynchronously), so gathers are issued as four per-queue chains of
SMALL (2-block) instructions with phase-shifted first sizes: small
instructions keep >1 instruction resident per descriptor ring, so descgen
runs back-to-back instead of stalling on the NX-decode ring-space await
(which also head-of-line blocks later queues). Per-core: 409600 tokens at
~2.0ns/token effective -> ~0.83ms gather phase.

Sharding (8 cores): batch-parallel gather (2048 rows/core); vocab-parallel s
precompute in bf16 (12800 padded rows/core) + bf16 AllGather, then a
one-shot SBUF-built spread + a DRAM write split across the two HWDGE
queues (never per-row descriptors).

Startup engineering (first gather at ~86us, was ~98us): the mlp gather-
ucode library is loaded at t=0 (hides the ~10us auto-inserted LOAD_LIB);
the table load is split across the Sync+Scalar HWDGE queues and ordered
ahead of the prefetches (s -> AllGather -> spread is the critical chain);
the DVE s-reduction is split (bf16 halves add at 2x rate + X-reduce over
D/2); mask-side inputs load on the Scalar queue; the last wave's masks are
hoisted into the ~40us collective wait (they otherwise serialize into the
post-gather tail).  The ~40us AllGather (~16us CC-stream launch latency +
ring hops) is the irreducible startup term; overlapping it with gathers of
local-vocab tokens was tried and measured SLOWER (requires shrinking the
gather blocks to 12 slots, which costs ~1.7ns/desc in descgen efficiency —
~150us total; see git history).

Host does layout only: shard/reshape inputs, compute j = idx>>4 / r = idx&15,
wrap indices in the dma_gather [16, S] layout, and concat per-core outputs.
"""

import os
import sys

import numpy as np

for _p in ("/opt/trn_rl_repo",):
    if os.path.isdir(_p) and _p not in sys.path:
        sys.path.insert(0, _p)

from concourse import bacc, bass, library_config, mybir, tile  # noqa: E402
from concourse.bass_utils import run_bass_kernel_spmd  # noqa: E402

F32 = mybir.dt.float32
BF16 = mybir.dt.bfloat16
I32 = mybir.dt.int32
I16 = mybir.dt.int16
P = 128
NCORES = 8


def dma_gather_raw(
    gp, out_ap, in_ap, idxs_ap, num_idxs, num_idxs_reg, elem_size, elem_step,
    queue_num=0, single_packet=False,
):
    """nc.gpsimd.dma_gather minus the 256B *element* restriction.

    Only the source row PITCH must be a 256B multiple (stride_bytes_256 is an
    8-bit field in 256B units); the per-index element payload can be smaller.
    Emits the same InstDMAGatherAnt the stock wrapper does.
    """
    dt_sz = mybir.dt.size(in_ap.dtype)
    stride_256 = (elem_step * dt_sz) // 256
    assert elem_step * dt_sz == stride_256 * 256 and 0 < stride_256 < 256
    assert in_ap.ap[0][0] == elem_step and in_ap.ap[-1][1] == elem_size
    _in_ap = gp.lower_ap_dma(in_ap, for_custom_bir_dma=True)
    _idxs_ap = gp.lower_ap(idxs_ap)
    _out_ap = gp.lower_ap(out_ap)
    return gp.add_instruction(
        mybir.InstDMAGatherAnt(
            name=gp.bass.get_next_instruction_name(),
            ins=[*_in_ap, _idxs_ap, gp.lower_val_access(gp.to_reg(num_idxs_reg))],
            outs=[_out_ap],
            transpose=False,
            num_idxs=num_idxs,
            elem_size=elem_size,
            stride_bytes_256=stride_256,
            gen_mode=0,
            single_packet=single_packet,
            queue_num=queue_num,
            sbuf_tokens_per_rank=0,
            sbuf_free_dim_per_rank=0,
            sbuf_free_dim_pad_per_rank=0,
            sbuf_byte_offset=0,
        )
    )


def build_program(
    G=16, L=200, D=100, RPP=98, CPI=100, NQ=4, ncores=NCORES, use_collective=True,
    repeat=1, ELEM=16, GAT_BUFS=12, SINGLE_PACKET=False, quad_split=False,
):
    """Build the SPMD program (identical on all cores).

    G:   row-groups per core (batch rows per core = G*128)
    L:   tokens per row
    D:   embedding dim
    RPP: padded vocab rows per SBUF partition (vocab rows per core = 128*RPP)
    CPI: token-slot BLOCK size (gather instructions cover 1..4 blocks);
         L % CPI == 0
    NQ:  SWDGE queues to rotate over (1..4)

    Gather instructions are grouped into per-queue chains whose first (and
    last) instructions have staggered sizes (1/2/3/4 blocks). Same-queue
    instructions serialize through descriptor-ring space, so equal-sized
    chains would fall into lockstep: all four queues' SDMA drains collide
    after each descgen wave and every queue idles through the combined
    drain. Staggered chain heads phase-shift the queues so each queue's
    drain overlaps the other queues' descgen.
    """
    assert L % CPI == 0
    BLK = CPI  # token slots per block
    SLOTS = G * L  # token slots per partition
    NBLK = SLOTS // BLK  # total blocks
    HB = L // BLK  # blocks per row-group
    ICOL = P * BLK // 16  # idxw columns per block
    # Max blocks per gather instruction. 2 is measured-optimal: at 2 blocks
    # (400 descs/engine) each queue's descriptor ring holds ~2.5
    # instructions and descgen runs back-to-back; at 3 blocks (600/engine,
    # <2 resident) descgen-drain lockstep returns and costs ~350us; at 4
    # it costs ~70us. Do not raise without re-measuring.
    MAXB = 2
    # Per-queue chains in blocks. Small instructions keep each queue's
    # descriptor ring holding >1 instruction, so Q7 descgen runs
    # back-to-back instead of stalling on the NX-decode ring-space await
    # (which also head-of-line blocks later queues' decode). Varied first
    # sizes phase-shift the queues.
    #
    # Equal per-queue split. (The trace shows queue 0 running gapless at
    # ~7.9ns/desc while queues 1-3 idle between ~23us instruction windows —
    # but rebalancing blocks toward queues 1-3 (17/37/37/37) measured
    # 1.26ms vs 0.94ms: the apparent fast-pair headroom is a dispatch
    # artifact, not spare descgen capacity. Keep the split equal.)
    if quad_split:
        # Phase-L: blocks {8g, 8g+1} of each row hold QUAD-local tokens
        # (vocab in this core's 4-core group slice) and gather from the
        # quad slab, which is ready after a fast 4-core AllGather; the
        # 8-core AllGather then completes under ~94us of phase-L gather
        # work. Global block numbering is unchanged so the reduce / idxw
        # / r2 slicing is identical. q0/q2 split their first L-pair into
        # 1-block instructions — the persistent half-instruction phase
        # offset that keeps the queues' SDMA drains de-phased.
        assert HB == 8 and MAXB == 2 and G % NQ == 0
        lch = {q: [] for q in range(NQ)}
        rch = {q: [] for q in range(NQ)}
        for g in range(G):
            q = g % NQ
            b0 = HB * g
            if g < NQ and q % 2 == 0:
                lch[q] += [(b0, 1), (b0 + 1, 1)]
            else:
                lch[q] += [(b0, 2)]
            rch[q] += [(b0 + 2, 2), (b0 + 4, 2), (b0 + 6, 2)]
        plan = []
        for part in (lch, rch):
            pos = {q: 0 for q in range(NQ)}
            while any(pos[q] < len(part[q]) for q in range(NQ)):
                for q in range(NQ):
                    if pos[q] < len(part[q]):
                        b0, n = part[q][pos[q]]
                        pos[q] += 1
                        plan.append((q, b0, n))
        NLINST = sum(len(v) for v in lch.values())
        assert sum(n for _, _, n in plan) == NBLK
    else:
        QBLOCKS = [NBLK // NQ] * NQ
        plan = []  # (queue, first block, nblocks), in issue order
        chains = []
        for q in range(NQ):
            first = (q % MAXB) + 1
            rest = QBLOCKS[q] - first
            sizes = [first] + [MAXB] * (rest // MAXB)
            if rest % MAXB:
                sizes.append(rest % MAXB)
            chains.append(sizes)
        blk0 = 0
        pos = [0] * NQ
        while any(pos[q] < len(chains[q]) for q in range(NQ)):
            for q in range(NQ):
                if pos[q] < len(chains[q]):
                    n = chains[q][pos[q]]
                    pos[q] += 1
                    plan.append((q, blk0, n))
                    blk0 += n
        assert blk0 == NBLK
        NLINST = 0
    VPC = P * RPP
    V_PAD = VPC * ncores
    # spread-table geometry: row j holds s[ELEM*j .. ELEM*j + ELEM) in the
    # first ELEM bf16 lanes of a 256B-pitch row. Wider rows shrink the
    # table (and its critical-path DRAM write) at the cost of a wider
    # DVE select, which is hidden under the gather waves.
    NROWS = V_PAD // ELEM
    assert NROWS % P == 0 and NROWS <= 32768
    nc = bacc.Bacc(
        "TRN2",
        target_bir_lowering=False,
        debug=False,
        num_devices=ncores,
        num_swdge_queues=NQ,
    )
    idxw_t = nc.dram_tensor("idxw", [P, SLOTS * 8], I16, kind="ExternalInput")
    io4_t = nc.dram_tensor("io4", [P, ELEM], BF16, kind="ExternalInput")
    r2_t = nc.dram_tensor("r2", [P, SLOTS], BF16, kind="ExternalInput")
    tab_t = nc.dram_tensor("tab", [P, RPP * D], BF16, kind="ExternalInput")
    w_t = nc.dram_tensor("w", [P, D], BF16, kind="ExternalInput")
    out_t = nc.dram_tensor("out", [P, G], F32, kind="ExternalOutput")

    with tile.TileContext(nc) as tc:
        with (
            tc.tile_pool(name="dr", bufs=1, space="DRAM") as dr,
            tc.tile_pool(name="keep", bufs=1) as keep,
        ):
            # Load the gather ucode library up front (gpsimd is idle during
            # startup); otherwise the auto-inserted LOAD_LIB lands right
            # before the first gather and costs ~10us on the critical path.
            nc.gpsimd.load_library(library_config.mlp)

            # ---- stage 1 loads first: the s -> collective -> spread chain
            # is the startup critical path, so the table halves go to the
            # two HWDGE queues (Sync + Scalar) before anything else.
            # w first on the Scalar queue (tiny, and the first-half mult
            # below needs it before the tab halves land)
            w_sb = keep.tile([P, D], BF16, name="w_sb")
            nc.scalar.dma_start(w_sb[:], w_t[:])
            tab_sb = keep.tile([P, RPP * D], BF16, name="tab_sb")
            # two-way split across the HWDGE queues (a third slice on the
            # gpsimd SWDGE queue was tried and lands LATE — it delays the
            # s-compute ~9us)
            HALF = RPP * D // 2
            nc.sync.dma_start(tab_sb[:, 0:HALF], tab_t[:, 0:HALF])
            nc.scalar.dma_start(tab_sb[:, HALF:], tab_t[:, HALF:])

            # ---- stage 0: prefetch gather-side inputs ----
            # idxw on Sync right behind the tab half (needed from ~90us);
            # mask-side inputs on Scalar.
            first_idxw = {}
            for q, b0, nb in plan[: NQ]:
                t_ = keep.tile([P, MAXB * ICOL], I16, name=f"idxwf{b0}")
                nc.sync.dma_start(
                    t_[:, 0 : nb * ICOL],
                    idxw_t[:, b0 * ICOL : (b0 + nb) * ICOL],
                )
                first_idxw[b0] = t_
            iota4 = keep.tile([P, ELEM], BF16)
            nc.scalar.dma_start(iota4[:], io4_t[:])
            r2_sb = keep.tile([P, SLOTS], BF16)
            nc.scalar.dma_start(r2_sb[:], r2_t[:])
            half_sb = keep.tile([P, NBLK], F32)
            out_sb = keep.tile([P, G], F32)

            with tc.tile_pool(name="pre", bufs=1) as pre:
                # ---- stage 1: s_part = (table slice) @ (w/L) ----
                # bf16 table/weights: halves the 5MB load and doubles DVE
                # throughput for the product; the reduce accumulates to f32.
                # Split the reduction: a bf16 tensor_tensor add of the two
                # D/2 halves (2x DVE rate) then an X-reduce over D/2 —
                # measured faster than one X-reduce over D (~10.6us -> ~7us).
                # mult split by ROW half so the first half starts as soon
                # as the Sync queue's tab half lands (~14.5us) instead of
                # waiting for both halves (~21us).
                prod_sb = pre.tile([P, RPP * D], BF16)
                R2_ = RPP // 2
                for h in range(2):
                    rs = slice(h * R2_, (h + 1) * R2_)
                    nc.vector.tensor_tensor(
                        out=prod_sb[:].rearrange("p (r d) -> p r d", d=D)[
                            :, rs, :
                        ],
                        in0=tab_sb[:].rearrange("p (r d) -> p r d", d=D)[
                            :, rs, :
                        ],
                        in1=w_sb[:].unsqueeze(1).to_broadcast([P, R2_, D]),
                        op=mybir.AluOpType.mult,
                    )
                D2 = D // 2
                psum_sb = pre.tile([P, RPP * D2], BF16)
                nc.vector.tensor_tensor(
                    out=psum_sb[:].rearrange("p (r d) -> p r d", d=D2),
                    in0=prod_sb[:].rearrange("p (r d) -> p r d", d=D)[:, :, 0:D2],
                    in1=prod_sb[:].rearrange("p (r d) -> p r d", d=D)[:, :, D2:D],
                    op=mybir.AluOpType.add,
                )
                s_sb = pre.tile([P, RPP], F32)
                nc.vector.tensor_reduce(
                    out=s_sb[:].unsqueeze(2),
                    in_=psum_sb[:].rearrange("p (r d) -> p r d", d=D2),
                    axis=mybir.AxisListType.X,
                    op=mybir.AluOpType.add,
                )

                # ---- stage 2: AllGather s (bf16) ----
                # The SWDGE write casts f32 -> bf16, halving the collective
                # payload (200KB).
                s_part = dr.tile([P, RPP], BF16)
                nc.gpsimd.dma_start(s_part[:], s_sb[:])
                RPW = NROWS // P  # spread rows per partition
                S16 = dr.tile([NROWS, 128], BF16)
                NQUAD = 4
                if quad_split:
                    assert use_collective
                    # fast 4-core AllGather first; the 8-core one (variable
                    # 40-88us: it waits on the slowest core) follows on the
                    # CC stream and completes under the phase-L gathers.
                    # Shared output: the non-Shared (local) out tile takes
                    # the CC bounce path (~230us to data-ready, measured).
                    # NOTE this assumes Shared scratchpad buffers are
                    # per-core copies — if they were one physical buffer
                    # the two quad groups would clobber each other (the
                    # rel-err gate catches that).
                    sq_full = dr.tile(
                        [NQUAD * RPP, P], BF16, addr_space="Shared"
                    )
                    nc.gpsimd.collective_compute(
                        "AllGather",
                        mybir.AluOpType.bypass,
                        replica_groups=[[0, 1, 2, 3], [4, 5, 6, 7]],
                        ins=[s_part.opt()],
                        outs=[sq_full.opt()],
                    )
                if use_collective:
                    s_full = dr.tile([ncores * RPP, P], BF16, addr_space="Shared")
                    nc.gpsimd.collective_compute(
                        "AllGather",
                        mybir.AluOpType.bypass,
                        replica_groups=[list(range(ncores))],
                        ins=[s_part.opt()],
                        outs=[s_full.opt()],
                    )
                else:
                    # crash-isolation mode: fill s_full with the local part
                    # replicated (wrong data, same program shape)
                    s_full = dr.tile([ncores * RPP, P], BF16)
                    for c in range(ncores):
                        nc.sync.dma_start(
                            s_full[c * RPP : (c + 1) * RPP, :],
                            s_part[:].rearrange("p r -> (p r)").rearrange(
                                "(r q) -> r q", q=P
                            ),
                        )

                # ---- stage 3a: quad spread (phase-L gather table) ----
                if quad_split:
                    NROWSQ = NQUAD * VPC // ELEM
                    RPWQ = NROWSQ // P
                    S16Q = dr.tile([NROWSQ, 128], BF16)
                    sqf_sb = pre.tile([P, RPWQ * ELEM], BF16)
                    # scalar queue: free after its startup loads; sync
                    # still feeds phase-L idxw tiles.
                    nc.scalar.dma_start(
                        sqf_sb[:],
                        sq_full[:]
                        .rearrange("a b -> (a b)")
                        .rearrange("(p x) -> p x", p=P),
                    )
                    sspq_sb = pre.tile([P, RPWQ * 128], BF16)
                    nc.vector.tensor_copy(
                        out=sspq_sb[:].rearrange("p (r k) -> p r k", k=128)[
                            :, :, 0:ELEM
                        ],
                        in_=sqf_sb[:].rearrange("p (r q) -> p r q", q=ELEM),
                    )
                    S16Qflat = S16Q[:].rearrange("a b -> (a b)").rearrange(
                        "(p x) -> p x", p=P
                    )
                    nc.scalar.dma_start(S16Qflat[:], sspq_sb[:])
                else:
                    S16Q = S16

            # Hoist the LAST wave's masks: they depend only on r2/io4
            # (landed ~20us) and otherwise serialize into the post-gather
            # tail. Emitted here so the DVE computes them during the
            # collective wait.
            full_plan = plan * repeat
            hoisted = {}
            for q, b0, nb in full_plan[-NQ:]:
                n = nb * BLK
                m_ = keep.tile([P, MAXB * BLK, ELEM], BF16, name=f"maskh{b0}")
                nc.vector.tensor_tensor(
                    out=m_[:, 0:n, :],
                    in0=r2_sb[:, b0 * BLK : b0 * BLK + n]
                    .unsqueeze(2)
                    .to_broadcast([P, n, ELEM]),
                    in1=iota4[:].unsqueeze(1).to_broadcast([P, n, ELEM]),
                    op=mybir.AluOpType.is_equal,
                )
                hoisted[b0] = m_

            with tc.tile_pool(name="gat", bufs=GAT_BUFS) as gat:
                # ---- stage 4: gather + select + reduce ----
                def emit_gather(i, q, b0, nb):
                    n = nb * BLK
                    # phase-L blocks (first 2 of each row) read the quad
                    # slab; everything else reads the full table.
                    src = S16Q if (quad_split and b0 % HB < MAXB) else S16
                    if i < NQ:
                        idxw_sb = first_idxw[b0]
                    else:
                        idxw_sb = gat.tile(
                            [P, MAXB * ICOL], I16, tag="idxw", name=f"idxw{b0}"
                        )
                        nc.sync.dma_start(
                            idxw_sb[:, 0 : nb * ICOL],
                            idxw_t[:, b0 * ICOL : (b0 + nb) * ICOL],
                        )
                    gth = gat.tile(
                        [P, MAXB * BLK, ELEM], BF16, tag="gth", name=f"gth{b0}"
                    )
                    dma_gather_raw(
                        nc.gpsimd,
                        gth[:, 0:n, :],
                        src[:, 0:ELEM],
                        idxw_sb[:, 0 : nb * ICOL],
                        P * n,
                        P * n,
                        elem_size=ELEM,
                        elem_step=128,
                        queue_num=q,
                        single_packet=SINGLE_PACKET,
                    )
                    if b0 in hoisted:
                        mask = hoisted[b0]
                    else:
                        mask = gat.tile(
                            [P, MAXB * BLK, ELEM], BF16, tag="mask", name=f"mask{b0}"
                        )
                        nc.vector.tensor_tensor(
                            out=mask[:, 0:n, :],
                            in0=r2_sb[:, b0 * BLK : b0 * BLK + n]
                            .unsqueeze(2)
                            .to_broadcast([P, n, ELEM]),
                            in1=iota4[:].unsqueeze(1).to_broadcast([P, n, ELEM]),
                            op=mybir.AluOpType.is_equal,
                        )
                    msel = gat.tile(
                        [P, MAXB * BLK, ELEM], BF16, tag="msel", name=f"msel{b0}"
                    )
                    nc.vector.tensor_tensor(
                        out=msel[:, 0:n, :],
                        in0=mask[:, 0:n, :],
                        in1=gth[:, 0:n, 0:ELEM],
                        op=mybir.AluOpType.mult,
                    )
                    nc.vector.tensor_reduce(
                        out=half_sb[:, b0 : b0 + nb].unsqueeze(2),
                        in_=msel[:, 0:n, :]
                        .rearrange("p a b -> p (a b)")
                        .rearrange("p (n x) -> p n x", x=BLK * ELEM),
                        axis=mybir.AxisListType.X,
                        op=mybir.AluOpType.add,
                    )

                for i, (q, b0, nb) in enumerate(full_plan[:NLINST]):
                    emit_gather(i, q, b0, nb)

                # ---- stage 3b: full-table spread, emitted AFTER phase-L
                # so the in-order DVE queue never blocks phase-L's
                # mask/select work on the (variable-latency) 8-core
                # AllGather. For quad_split=False, NLINST=0 and this is
                # the original pre-gather position.
                sfull_sb = keep.tile([P, RPW * ELEM], BF16, name="sfull_sb")
                nc.sync.dma_start(
                    sfull_sb[:],
                    s_full[:]
                    .rearrange("a b -> (a b)")
                    .rearrange("(p x) -> p x", p=P),
                )
                ssp_sb = keep.tile([P, RPW * 128], BF16, name="ssp_sb")
                nc.vector.tensor_copy(
                    out=ssp_sb[:].rearrange("p (r k) -> p r k", k=128)[
                        :, :, 0:ELEM
                    ],
                    in_=sfull_sb[:].rearrange("p (r q) -> p r q", q=ELEM),
                )
                S16flat = S16[:].rearrange("a b -> (a b)").rearrange(
                    "(p x) -> p x", p=P
                )
                XH = RPW * 128 // 2
                nc.sync.dma_start(S16flat[:, 0:XH], ssp_sb[:, 0:XH])
                nc.scalar.dma_start(S16flat[:, XH:], ssp_sb[:, XH:])

                for j, (q, b0, nb) in enumerate(full_plan[NLINST:]):
                    emit_gather(NLINST + j, q, b0, nb)
                nc.vector.tensor_reduce(
                    out=out_sb[:].unsqueeze(2),
                    in_=half_sb[:].rearrange("p (g h) -> p g h", h=HB),
                    axis=mybir.AxisListType.X,
                    op=mybir.AluOpType.add,
                )
                nc.sync.dma_start(out_t[:], out_sb[:])
    nc.compile()
    return nc


def make_in_maps(
    word_idxs, embed_table, weights, G, L, D, RPP, CPI, ncores=NCORES,
    quad_split=False,
):
    """Shard + lay out the full inputs for the per-core program."""
    BPC = G * P
    SLOTS = G * L
    NT = SLOTS // CPI
    VPC = P * RPP
    import ml_dtypes

    bf16 = ml_dtypes.bfloat16
    idx = np.asarray(word_idxs).astype(np.int32)
    tab = np.asarray(embed_table, dtype=np.float32)
    w = np.asarray(weights, dtype=np.float32).reshape(-1)
    V = tab.shape[0]
    tab_pad = np.zeros((VPC * ncores, D), dtype=bf16)
    tab_pad[:V] = tab.astype(bf16)
    w_c = np.ascontiguousarray(
        np.broadcast_to((w / np.float32(L))[None, :].astype(bf16), (P, D))
    )
    in_maps = []
    for c in range(ncores):
        # token slot layout: [partition p, slot j=g*L+l] holds idx of batch
        # row (c*BPC + g*128 + p), token l
        rows = idx[c * BPC : (c + 1) * BPC]  # [BPC, L]
        if quad_split:
            # quad-local tokens first: the first 2*CPI slots of each row
            # are gathered from the quad slab (phase-L)
            Q = c // 4
            is_q = (rows // (4 * VPC)) == Q
            assert is_q.sum(axis=1).min() >= 2 * CPI
            order = np.argsort(~is_q, axis=1, kind="stable")
            rows = np.take_along_axis(rows, order, axis=1)
        slots = rows.reshape(G, P, L).transpose(1, 0, 2).reshape(P, SLOTS)
        jmat = (slots >> 4).astype(np.int16)  # [P, SLOTS]
        r2 = (slots & 15).astype(bf16)
        if quad_split:
            # phase-L slots index the quad slab (lane = v & 15 unchanged:
            # 4*VPC is a multiple of 16)
            QL = 2 * CPI
            sl3 = slots.reshape(P, G, L)
            j3 = jmat.reshape(P, G, L)
            j3[:, :, 0:QL] = (
                (sl3[:, :, 0:QL] - (c // 4) * 4 * VPC) >> 4
            ).astype(np.int16)
        # per-instruction index lists in i = c_local*128 + p order, wrapped
        # into the dma_gather [16, NI//16] layout, replicated to 128 parts
        u = jmat.reshape(P, NT, CPI).transpose(1, 2, 0)  # [NT, CPI, P]
        wrp = u.reshape(NT, CPI * P // 16, 16).transpose(2, 0, 1).reshape(16, -1)
        idxw = np.ascontiguousarray(np.tile(wrp, (8, 1)))  # [128, SLOTS*8]
        tab_c = np.ascontiguousarray(
            tab_pad[c * VPC : (c + 1) * VPC].reshape(P, RPP * D)
        )
        in_maps.append(
            {
                "idxw": idxw,
                "r2": np.ascontiguousarray(r2),
                "tab": tab_c,
                "w": w_c,
                "io4": np.ascontiguousarray(
                    np.broadcast_to(
                        np.arange(16, dtype=np.float32).astype(bf16), (P, 16)
                    )
                ),
            }
        )
    return in_maps


def unshard_out(results, G, ncores=NCORES):
    """results: list of per-core {'out': [128, G]} -> full [B, 1] f32."""
    parts = []
    for c in range(ncores):
        o = np.asarray(results[c]["out"])  # [P, G]; out[p, g] = row g*128+p
        parts.append(o.T.reshape(-1))
    return np.concatenate(parts).reshape(-1, 1).astype(np.float32)


_CACHED_NC = None

FULL = dict(G=16, L=200, D=100, RPP=100, CPI=25)


def _quad_ok(idx):
    """quad_split needs >= 2*CPI quad-local tokens in EVERY row."""
    BPC = FULL["G"] * P
    qvpc = 4 * P * FULL["RPP"]
    for c in range(NCORES):
        rows = idx[c * BPC : (c + 1) * BPC]
        if ((rows // qvpc) == (c // 4)).sum(axis=1).min() < 2 * FULL["CPI"]:
            return False
    return True


def _get_nc(quad_split):
    global _CACHED_NC
    if _CACHED_NC is None or _CACHED_NC[1] != quad_split:
        _CACHED_NC = (build_program(**FULL, quad_split=quad_split), quad_split)
    return _CACHED_NC[0]


def run(word_idxs, embed_table, weights, trace=False, **spmd_kwargs):
    """Build (cached), run on the 8 cores, return (full_out, BassKernelResults)."""
    idx = np.asarray(word_idxs).astype(np.int32)
    # quad_split is dead on this stack: a 4-core-group AllGather cannot
    # use the Shared-output fast path ("needs >4 cores") and the local-out
    # bounce path is ~230us to data-ready (measured 1175us total); an
    # 8-core prefix collective would pay the same ~40-88us latency as the
    # full one (latency, not payload, dominates). Keep False.
    quad_split = False and _quad_ok(idx)
    nc = _get_nc(quad_split)
    in_maps = make_in_maps(
        idx,
        embed_table,
        weights,
        FULL["G"],
        FULL["L"],
        FULL["D"],
        FULL["RPP"],
        FULL["CPI"],
        quad_split=quad_split,
    )
    res = run_bass_kernel_spmd(
        nc, in_maps, core_ids=list(range(NCORES)), trace=trace, **spmd_kwargs
    )
    out = unshard_out(res.results, FULL["G"])
    return out, res


def kernel(word_idxs, embed_table, weights):
    out, _ = run(word_idxs, embed_table, weights, trace=False)
    return out



# revision 29
# speedup vs baseline: 1.1918x; 1.0089x over previous
"""Trainium2 Bass kernel for fused embedding-lookup -> mean-pool -> dot(weights).

Reference computation (B=16384, L=200, D=100, V=100000):
    out[b] = mean_l(embed_table[word_idxs[b, l], :]) @ weights            # [B, 1]

Key algebraic transform: the dot with `weights` is linear, so
    out[b] = sum_l s[word_idxs[b, l]],   with  s = embed_table @ (weights / L)
Instead of gathering B*L rows of 400B (1.31 GB), we precompute the V-element
vector `s` on-device (the 40MB table is read exactly once across the 8 cores)
and gather B*L scalars.

The scalar gather uses the TIE-ucode `dma_gather` (int16 row indices). Its
row stride must be a 256B multiple, so s is spread into a bf16 table
    S16[j, 0:16] = s_pad[16j .. 16j+16),  row pitch 128 bf16 = 256B,
and a token with index v gathers row j = v >> 4 (32B payload), after which
a 16-wide mask+reduce selects lane r = v & 15 and accumulates per
row-group. (16 values per row keeps the table write at 1.6MB / ~4.5us on
the startup critical path; the wider DVE select hides under the gather.)

Performance structure (HW-measured): the wall is Q7 descriptor GENERATION
inside the dma_gather ucode — each gather runs on one Q7 core pair (pair =
queue_num) at ~9.4 cycles/descriptor, one descriptor per token. All four
queue pairs generate concurrently (the 8 Q7 cores pop the NX instruction
queue asynchronously), so gathers are issued as four per-queue chains of
SMALL (2-block) instructions with phase-shifted first sizes: small
instructions keep >1 instruction resident per descriptor ring, so descgen
runs back-to-back instead of stalling on the NX-decode ring-space await
(which also head-of-line blocks later queues). Per-core: 409600 tokens at
~2.0ns/token effective -> ~0.83ms gather phase.

Sharding (8 cores): batch-parallel gather (2048 rows/core); vocab-parallel s
precompute in bf16 (12800 padded rows/core) + bf16 AllGather, then a
one-shot SBUF-built spread + a DRAM write split across the two HWDGE
queues (never per-row descriptors).

Startup engineering (first gather at ~86us, was ~98us): the mlp gather-
ucode library is loaded at t=0 (hides the ~10us auto-inserted LOAD_LIB);
the table load is split across the Sync+Scalar HWDGE queues and ordered
ahead of the prefetches (s -> AllGather -> spread is the critical chain);
the DVE s-reduction is split (bf16 halves add at 2x rate + X-reduce over
D/2); mask-side inputs load on the Scalar queue; the last wave's masks are
hoisted into the ~40us collective wait (they otherwise serialize into the
post-gather tail).  The ~40us AllGather (~16us CC-stream launch latency +
ring hops) is the irreducible startup term; overlapping it with gathers of
local-vocab tokens was tried and measured SLOWER (requires shrinking the
gather blocks to 12 slots, which costs ~1.7ns/desc in descgen efficiency —
~150us total; see git history).

Host does layout only: shard/reshape inputs, compute j = idx>>4 / r = idx&15,
wrap indices in the dma_gather [16, S] layout, and concat per-core outputs.
"""

import os
import sys

import numpy as np

for _p in ("/opt/trn_rl_repo",):
    if os.path.isdir(_p) and _p not in sys.path:
        sys.path.insert(0, _p)

from concourse import bacc, bass, library_config, mybir, tile  # noqa: E402
from concourse.bass_utils import run_bass_kernel_spmd  # noqa: E402

F32 = mybir.dt.float32
BF16 = mybir.dt.bfloat16
I32 = mybir.dt.int32
I16 = mybir.dt.int16
P = 128
NCORES = 8


def dma_gather_raw(
    gp, out_ap, in_ap, idxs_ap, num_idxs, num_idxs_reg, elem_size, elem_step,
    queue_num=0, single_packet=False,
):
    """nc.gpsimd.dma_gather minus the 256B *element* restriction.

    Only the source row PITCH must be a 256B multiple (stride_bytes_256 is an
    8-bit field in 256B units); the per-index element payload can be smaller.
    Emits the same InstDMAGatherAnt the stock wrapper does.
    """
    dt_sz = mybir.dt.size(in_ap.dtype)
    stride_256 = (elem_step * dt_sz) // 256
    assert elem_step * dt_sz == stride_256 * 256 and 0 < stride_256 < 256
    assert in_ap.ap[0][0] == elem_step and in_ap.ap[-1][1] == elem_size
    _in_ap = gp.lower_ap_dma(in_ap, for_custom_bir_dma=True)
    _idxs_ap = gp.lower_ap(idxs_ap)
    _out_ap = gp.lower_ap(out_ap)
    return gp.add_instruction(
        mybir.InstDMAGatherAnt(
            name=gp.bass.get_next_instruction_name(),
            ins=[*_in_ap, _idxs_ap, gp.lower_val_access(gp.to_reg(num_idxs_reg))],
            outs=[_out_ap],
            transpose=False,
            num_idxs=num_idxs,
            elem_size=elem_size,
            stride_bytes_256=stride_256,
            gen_mode=0,
            single_packet=single_packet,
            queue_num=queue_num,
            sbuf_tokens_per_rank=0,
            sbuf_free_dim_per_rank=0,
            sbuf_free_dim_pad_per_rank=0,
            sbuf_byte_offset=0,
        )
    )


def build_program(
    G=16, L=200, D=100, RPP=98, CPI=100, NQ=4, ncores=NCORES, use_collective=True,
    repeat=1, ELEM=16, GAT_BUFS=12, SINGLE_PACKET=False, quad_split=False,
):
    """Build the SPMD program (identical on all cores).

    G:   row-groups per core (batch rows per core = G*128)
    L:   tokens per row
    D:   embedding dim
    RPP: padded vocab rows per SBUF partition (vocab rows per core = 128*RPP)
    CPI: token-slot BLOCK size (gather instructions cover 1..4 blocks);
         L % CPI == 0
    NQ:  SWDGE queues to rotate over (1..4)

    Gather instructions are grouped into per-queue chains whose first (and
    last) instructions have staggered sizes (1/2/3/4 blocks). Same-queue
    instructions serialize through descriptor-ring space, so equal-sized
    chains would fall into lockstep: all four queues' SDMA drains collide
    after each descgen wave and every queue idles through the combined
    drain. Staggered chain heads phase-shift the queues so each queue's
    drain overlaps the other queues' descgen.
    """
    assert L % CPI == 0
    BLK = CPI  # token slots per block
    SLOTS = G * L  # token slots per partition
    NBLK = SLOTS // BLK  # total blocks
    HB = L // BLK  # blocks per row-group
    ICOL = P * BLK // 16  # idxw columns per block
    # Max blocks per gather instruction. 2 is measured-optimal: at 2 blocks
    # (400 descs/engine) each queue's descriptor ring holds ~2.5
    # instructions and descgen runs back-to-back; at 3 blocks (600/engine,
    # <2 resident) descgen-drain lockstep returns and costs ~350us; at 4
    # it costs ~70us. Do not raise without re-measuring.
    MAXB = 2
    # Per-queue chains in blocks. Small instructions keep each queue's
    # descriptor ring holding >1 instruction, so Q7 descgen runs
    # back-to-back instead of stalling on the NX-decode ring-space await
    # (which also head-of-line blocks later queues' decode). Varied first
    # sizes phase-shift the queues.
    #
    # Equal per-queue split. (The trace shows queue 0 running gapless at
    # ~7.9ns/desc while queues 1-3 idle between ~23us instruction windows —
    # but rebalancing blocks toward queues 1-3 (17/37/37/37) measured
    # 1.26ms vs 0.94ms: the apparent fast-pair headroom is a dispatch
    # artifact, not spare descgen capacity. Keep the split equal.)
    if quad_split:
        # Phase-L: blocks {8g, 8g+1} of each row hold QUAD-local tokens
        # (vocab in this core's 4-core group slice) and gather from the
        # quad slab, which is ready after a fast 4-core AllGather; the
        # 8-core AllGather then completes under ~94us of phase-L gather
        # work. Global block numbering is unchanged so the reduce / idxw
        # / r2 slicing is identical. q0/q2 split their first L-pair into
        # 1-block instructions — the persistent half-instruction phase
        # offset that keeps the queues' SDMA drains de-phased.
        assert HB == 8 and MAXB == 2 and G % NQ == 0
        lch = {q: [] for q in range(NQ)}
        rch = {q: [] for q in range(NQ)}
        for g in range(G):
            q = g % NQ
            b0 = HB * g
            if g < NQ and q % 2 == 0:
                lch[q] += [(b0, 1), (b0 + 1, 1)]
            else:
                lch[q] += [(b0, 2)]
            rch[q] += [(b0 + 2, 2), (b0 + 4, 2), (b0 + 6, 2)]
        plan = []
        for part in (lch, rch):
            pos = {q: 0 for q in range(NQ)}
            while any(pos[q] < len(part[q]) for q in range(NQ)):
                for q in range(NQ):
                    if pos[q] < len(part[q]):
                        b0, n = part[q][pos[q]]
                        pos[q] += 1
                        plan.append((q, b0, n))
        NLINST = sum(len(v) for v in lch.values())
        assert sum(n for _, _, n in plan) == NBLK
    else:
        QBLOCKS = [NBLK // NQ] * NQ
        plan = []  # (queue, first block, nblocks), in issue order
        chains = []
        for q in range(NQ):
            first = (q % MAXB) + 1
            rest = QBLOCKS[q] - first
            sizes = [first] + [MAXB] * (rest // MAXB)
            if rest % MAXB:
                sizes.append(rest % MAXB)
            chains.append(sizes)
        blk0 = 0
        pos = [0] * NQ
        while any(pos[q] < len(chains[q]) for q in range(NQ)):
            for q in range(NQ):
                if pos[q] < len(chains[q]):
                    n = chains[q][pos[q]]
                    pos[q] += 1
                    plan.append((q, blk0, n))
                    blk0 += n
        assert blk0 == NBLK
        NLINST = 0
    VPC = P * RPP
    V_PAD = VPC * ncores
    # spread-table geometry: row j holds s[ELEM*j .. ELEM*j + ELEM) in the
    # first ELEM bf16 lanes of a 256B-pitch row. Wider rows shrink the
    # table (and its critical-path DRAM write) at the cost of a wider
    # DVE select, which is hidden under the gather waves.
    NROWS = V_PAD // ELEM
    assert NROWS % P == 0 and NROWS <= 32768
    nc = bacc.Bacc(
        "TRN2",
        target_bir_lowering=False,
        debug=False,
        num_devices=ncores,
        num_swdge_queues=NQ,
    )
    idxw_t = nc.dram_tensor("idxw", [P, SLOTS * 8], I16, kind="ExternalInput")
    io4_t = nc.dram_tensor("io4", [P, ELEM], BF16, kind="ExternalInput")
    r2_t = nc.dram_tensor("r2", [P, SLOTS], BF16, kind="ExternalInput")
    tab_t = nc.dram_tensor("tab", [P, RPP * D], BF16, kind="ExternalInput")
    w_t = nc.dram_tensor("w", [P, D], BF16, kind="ExternalInput")
    out_t = nc.dram_tensor("out", [P, G], F32, kind="ExternalOutput")

    with tile.TileContext(nc) as tc:
        with (
            tc.tile_pool(name="dr", bufs=1, space="DRAM") as dr,
            tc.tile_pool(name="keep", bufs=1) as keep,
        ):
            # Load the gather ucode library up front (gpsimd is idle during
            # startup); otherwise the auto-inserted LOAD_LIB lands right
            # before the first gather and costs ~10us on the critical path.
            nc.gpsimd.load_library(library_config.mlp)

            # ---- stage 1 loads first: the s -> collective -> spread chain
            # is the startup critical path, so the table halves go to the
            # two HWDGE queues (Sync + Scalar) before anything else.
            # w first on the Scalar queue (tiny, and the first-half mult
            # below needs it before the tab halves land)
            w_sb = keep.tile([P, D], BF16, name="w_sb")
            nc.scalar.dma_start(w_sb[:], w_t[:])
            tab_sb = keep.tile([P, RPP * D], BF16, name="tab_sb")
            # two-way split across the HWDGE queues (a third slice on the
            # gpsimd SWDGE queue was tried and lands LATE — it delays the
            # s-compute ~9us)
            HALF = RPP * D // 2
            nc.sync.dma_start(tab_sb[:, 0:HALF], tab_t[:, 0:HALF])
            nc.scalar.dma_start(tab_sb[:, HALF:], tab_t[:, HALF:])

            # ---- stage 0: prefetch gather-side inputs ----
            # idxw on Sync right behind the tab half (needed from ~90us);
            # mask-side inputs on Scalar.
            first_idxw = {}
            for q, b0, nb in plan[: NQ]:
                t_ = keep.tile([P, MAXB * ICOL], I16, name=f"idxwf{b0}")
                nc.sync.dma_start(
                    t_[:, 0 : nb * ICOL],
                    idxw_t[:, b0 * ICOL : (b0 + nb) * ICOL],
                )
                first_idxw[b0] = t_
            iota4 = keep.tile([P, ELEM], BF16)
            nc.scalar.dma_start(iota4[:], io4_t[:])
            r2_sb = keep.tile([P, SLOTS], BF16)
            nc.scalar.dma_start(r2_sb[:], r2_t[:])
            half_sb = keep.tile([P, NBLK], F32)
            out_sb = keep.tile([P, G], F32)

            with tc.tile_pool(name="pre", bufs=1) as pre:
                # ---- stage 1: s_part = (table slice) @ (w/L) ----
                # bf16 table/weights: halves the 5MB load and doubles DVE
                # throughput for the product; the reduce accumulates to f32.
                # Split the reduction: a bf16 tensor_tensor add of the two
                # D/2 halves (2x DVE rate) then an X-reduce over D/2 —
                # measured faster than one X-reduce over D (~10.6us -> ~7us).
                # The whole mult -> halves-add -> X-reduce -> bf16-store
                # chain is split by ROW half: half 0's chain runs entirely
                # during the Scalar queue's tab-half load, so the collective
                # trigger moves ~7-8us earlier. Numerically identical.
                prod_sb = pre.tile([P, RPP * D], BF16)
                D2 = D // 2
                psum_sb = pre.tile([P, RPP * D2], BF16)
                s_sb = pre.tile([P, RPP], F32)
                s_part = dr.tile([P, RPP], BF16)
                R2_ = RPP // 2
                for h in range(2):
                    rs = slice(h * R2_, (h + 1) * R2_)
                    nc.vector.tensor_tensor(
                        out=prod_sb[:].rearrange("p (r d) -> p r d", d=D)[
                            :, rs, :
                        ],
                        in0=tab_sb[:].rearrange("p (r d) -> p r d", d=D)[
                            :, rs, :
                        ],
                        in1=w_sb[:].unsqueeze(1).to_broadcast([P, R2_, D]),
                        op=mybir.AluOpType.mult,
                    )
                    nc.vector.tensor_tensor(
                        out=psum_sb[:].rearrange("p (r d) -> p r d", d=D2)[
                            :, rs, :
                        ],
                        in0=prod_sb[:].rearrange("p (r d) -> p r d", d=D)[
                            :, rs, 0:D2
                        ],
                        in1=prod_sb[:].rearrange("p (r d) -> p r d", d=D)[
                            :, rs, D2:D
                        ],
                        op=mybir.AluOpType.add,
                    )
                    nc.vector.tensor_reduce(
                        out=s_sb[:, rs].unsqueeze(2),
                        in_=psum_sb[:].rearrange("p (r d) -> p r d", d=D2)[
                            :, rs, :
                        ],
                        axis=mybir.AxisListType.X,
                        op=mybir.AluOpType.add,
                    )
                    # ---- stage 2 feed: the SWDGE store casts f32 -> bf16;
                    # half 0's store overlaps half 1's compute.
                    nc.gpsimd.dma_start(s_part[:, rs], s_sb[:, rs])
                RPW = NROWS // P  # spread rows per partition
                S16 = dr.tile([NROWS, 128], BF16)
                NQUAD = 4
                if quad_split:
                    assert use_collective
                    # fast 4-core AllGather first; the 8-core one (variable
                    # 40-88us: it waits on the slowest core) follows on the
                    # CC stream and completes under the phase-L gathers.
                    # Shared output: the non-Shared (local) out tile takes
                    # the CC bounce path (~230us to data-ready, measured).
                    # NOTE this assumes Shared scratchpad buffers are
                    # per-core copies — if they were one physical buffer
                    # the two quad groups would clobber each other (the
                    # rel-err gate catches that).
                    sq_full = dr.tile(
                        [NQUAD * RPP, P], BF16, addr_space="Shared"
                    )
                    nc.gpsimd.collective_compute(
                        "AllGather",
                        mybir.AluOpType.bypass,
                        replica_groups=[[0, 1, 2, 3], [4, 5, 6, 7]],
                        ins=[s_part.opt()],
                        outs=[sq_full.opt()],
                    )
                if use_collective:
                    s_full = dr.tile([ncores * RPP, P], BF16, addr_space="Shared")
                    nc.gpsimd.collective_compute(
                        "AllGather",
                        mybir.AluOpType.bypass,
                        replica_groups=[list(range(ncores))],
                        ins=[s_part.opt()],
                        outs=[s_full.opt()],
                    )
                else:
                    # crash-isolation mode: fill s_full with the local part
                    # replicated (wrong data, same program shape)
                    s_full = dr.tile([ncores * RPP, P], BF16)
                    for c in range(ncores):
                        nc.sync.dma_start(
                            s_full[c * RPP : (c + 1) * RPP, :],
                            s_part[:].rearrange("p r -> (p r)").rearrange(
                                "(r q) -> r q", q=P
                            ),
                        )

                # ---- stage 3a: quad spread (phase-L gather table) ----
                if quad_split:
                    NROWSQ = NQUAD * VPC // ELEM
                    RPWQ = NROWSQ // P
                    S16Q = dr.tile([NROWSQ, 128], BF16)
                    sqf_sb = pre.tile([P, RPWQ * ELEM], BF16)
                    # scalar queue: free after its startup loads; sync
                    # still feeds phase-L idxw tiles.
                    nc.scalar.dma_start(
                        sqf_sb[:],
                        sq_full[:]
                        .rearrange("a b -> (a b)")
                        .rearrange("(p x) -> p x", p=P),
                    )
                    sspq_sb = pre.tile([P, RPWQ * 128], BF16)
                    nc.vector.tensor_copy(
                        out=sspq_sb[:].rearrange("p (r k) -> p r k", k=128)[
                            :, :, 0:ELEM
                        ],
                        in_=sqf_sb[:].rearrange("p (r q) -> p r q", q=ELEM),
                    )
                    S16Qflat = S16Q[:].rearrange("a b -> (a b)").rearrange(
                        "(p x) -> p x", p=P
                    )
                    nc.scalar.dma_start(S16Qflat[:], sspq_sb[:])
                else:
                    S16Q = S16

            # Hoist the LAST wave's masks: they depend only on r2/io4
            # (landed ~20us) and otherwise serialize into the post-gather
            # tail. Emitted here so the DVE computes them during the
            # collective wait.
            full_plan = plan * repeat
            hoisted = {}
            for q, b0, nb in full_plan[-NQ:]:
                n = nb * BLK
                m_ = keep.tile([P, MAXB * BLK, ELEM], BF16, name=f"maskh{b0}")
                nc.vector.tensor_tensor(
                    out=m_[:, 0:n, :],
                    in0=r2_sb[:, b0 * BLK : b0 * BLK + n]
                    .unsqueeze(2)
                    .to_broadcast([P, n, ELEM]),
                    in1=iota4[:].unsqueeze(1).to_broadcast([P, n, ELEM]),
                    op=mybir.AluOpType.is_equal,
                )
                hoisted[b0] = m_

            with tc.tile_pool(name="gat", bufs=GAT_BUFS) as gat:
                # ---- stage 4: gather + select + reduce ----
                def emit_gather(i, q, b0, nb):
                    n = nb * BLK
                    # phase-L blocks (first 2 of each row) read the quad
                    # slab; everything else reads the full table.
                    src = S16Q if (quad_split and b0 % HB < MAXB) else S16
                    if i < NQ:
                        idxw_sb = first_idxw[b0]
                    else:
                        idxw_sb = gat.tile(
                            [P, MAXB * ICOL], I16, tag="idxw", name=f"idxw{b0}"
                        )
                        nc.sync.dma_start(
                            idxw_sb[:, 0 : nb * ICOL],
                            idxw_t[:, b0 * ICOL : (b0 + nb) * ICOL],
                        )
                    gth = gat.tile(
                        [P, MAXB * BLK, ELEM], BF16, tag="gth", name=f"gth{b0}"
                    )
                    dma_gather_raw(
                        nc.gpsimd,
                        gth[:, 0:n, :],
                        src[:, 0:ELEM],
                        idxw_sb[:, 0 : nb * ICOL],
                        P * n,
                        P * n,
                        elem_size=ELEM,
                        elem_step=128,
                        queue_num=q,
                        single_packet=SINGLE_PACKET,
                    )
                    if b0 in hoisted:
                        mask = hoisted[b0]
                    else:
                        mask = gat.tile(
                            [P, MAXB * BLK, ELEM], BF16, tag="mask", name=f"mask{b0}"
                        )
                        nc.vector.tensor_tensor(
                            out=mask[:, 0:n, :],
                            in0=r2_sb[:, b0 * BLK : b0 * BLK + n]
                            .unsqueeze(2)
                            .to_broadcast([P, n, ELEM]),
                            in1=iota4[:].unsqueeze(1).to_broadcast([P, n, ELEM]),
                            op=mybir.AluOpType.is_equal,
                        )
                    msel = gat.tile(
                        [P, MAXB * BLK, ELEM], BF16, tag="msel", name=f"msel{b0}"
                    )
                    nc.vector.tensor_tensor(
                        out=msel[:, 0:n, :],
                        in0=mask[:, 0:n, :],
                        in1=gth[:, 0:n, 0:ELEM],
                        op=mybir.AluOpType.mult,
                    )
                    nc.vector.tensor_reduce(
                        out=half_sb[:, b0 : b0 + nb].unsqueeze(2),
                        in_=msel[:, 0:n, :]
                        .rearrange("p a b -> p (a b)")
                        .rearrange("p (n x) -> p n x", x=BLK * ELEM),
                        axis=mybir.AxisListType.X,
                        op=mybir.AluOpType.add,
                    )

                for i, (q, b0, nb) in enumerate(full_plan[:NLINST]):
                    emit_gather(i, q, b0, nb)

                # ---- stage 3b: full-table spread, emitted AFTER phase-L
                # so the in-order DVE queue never blocks phase-L's
                # mask/select work on the (variable-latency) 8-core
                # AllGather. For quad_split=False, NLINST=0 and this is
                # the original pre-gather position.
                sfull_sb = keep.tile([P, RPW * ELEM], BF16, name="sfull_sb")
                nc.sync.dma_start(
                    sfull_sb[:],
                    s_full[:]
                    .rearrange("a b -> (a b)")
                    .rearrange("(p x) -> p x", p=P),
                )
                ssp_sb = keep.tile([P, RPW * 128], BF16, name="ssp_sb")
                nc.vector.tensor_copy(
                    out=ssp_sb[:].rearrange("p (r k) -> p r k", k=128)[
                        :, :, 0:ELEM
                    ],
                    in_=sfull_sb[:].rearrange("p (r q) -> p r q", q=ELEM),
                )
                S16flat = S16[:].rearrange("a b -> (a b)").rearrange(
                    "(p x) -> p x", p=P
                )
                XH = RPW * 128 // 2
                nc.sync.dma_start(S16flat[:, 0:XH], ssp_sb[:, 0:XH])
                nc.scalar.dma_start(S16flat[:, XH:], ssp_sb[:, XH:])

                for j, (q, b0, nb) in enumerate(full_plan[NLINST:]):
                    emit_gather(NLINST + j, q, b0, nb)
                nc.vector.tensor_reduce(
                    out=out_sb[:].unsqueeze(2),
                    in_=half_sb[:].rearrange("p (g h) -> p g h", h=HB),
                    axis=mybir.AxisListType.X,
                    op=mybir.AluOpType.add,
                )
                nc.sync.dma_start(out_t[:], out_sb[:])
    nc.compile()
    return nc


def make_in_maps(
    word_idxs, embed_table, weights, G, L, D, RPP, CPI, ncores=NCORES,
    quad_split=False,
):
    """Shard + lay out the full inputs for the per-core program."""
    BPC = G * P
    SLOTS = G * L
    NT = SLOTS // CPI
    VPC = P * RPP
    import ml_dtypes

    bf16 = ml_dtypes.bfloat16
    idx = np.asarray(word_idxs).astype(np.int32)
    tab = np.asarray(embed_table, dtype=np.float32)
    w = np.asarray(weights, dtype=np.float32).reshape(-1)
    V = tab.shape[0]
    tab_pad = np.zeros((VPC * ncores, D), dtype=bf16)
    tab_pad[:V] = tab.astype(bf16)
    w_c = np.ascontiguousarray(
        np.broadcast_to((w / np.float32(L))[None, :].astype(bf16), (P, D))
    )
    in_maps = []
    for c in range(ncores):
        # token slot layout: [partition p, slot j=g*L+l] holds idx of batch
        # row (c*BPC + g*128 + p), token l
        rows = idx[c * BPC : (c + 1) * BPC]  # [BPC, L]
        if quad_split:
            # quad-local tokens first: the first 2*CPI slots of each row
            # are gathered from the quad slab (phase-L)
            Q = c // 4
            is_q = (rows // (4 * VPC)) == Q
            assert is_q.sum(axis=1).min() >= 2 * CPI
            order = np.argsort(~is_q, axis=1, kind="stable")
            rows = np.take_along_axis(rows, order, axis=1)
        slots = rows.reshape(G, P, L).transpose(1, 0, 2).reshape(P, SLOTS)
        jmat = (slots >> 4).astype(np.int16)  # [P, SLOTS]
        r2 = (slots & 15).astype(bf16)
        if quad_split:
            # phase-L slots index the quad slab (lane = v & 15 unchanged:
            # 4*VPC is a multiple of 16)
            QL = 2 * CPI
            sl3 = slots.reshape(P, G, L)
            j3 = jmat.reshape(P, G, L)
            j3[:, :, 0:QL] = (
                (sl3[:, :, 0:QL] - (c // 4) * 4 * VPC) >> 4
            ).astype(np.int16)
        # per-instruction index lists in i = c_local*128 + p order, wrapped
        # into the dma_gather [16, NI//16] layout, replicated to 128 parts
        u = jmat.reshape(P, NT, CPI).transpose(1, 2, 0)  # [NT, CPI, P]
        wrp = u.reshape(NT, CPI * P // 16, 16).transpose(2, 0, 1).reshape(16, -1)
        idxw = np.ascontiguousarray(np.tile(wrp, (8, 1)))  # [128, SLOTS*8]
        tab_c = np.ascontiguousarray(
            tab_pad[c * VPC : (c + 1) * VPC].reshape(P, RPP * D)
        )
        in_maps.append(
            {
                "idxw": idxw,
                "r2": np.ascontiguousarray(r2),
                "tab": tab_c,
                "w": w_c,
                "io4": np.ascontiguousarray(
                    np.broadcast_to(
                        np.arange(16, dtype=np.float32).astype(bf16), (P, 16)
                    )
                ),
            }
        )
    return in_maps


def unshard_out(results, G, ncores=NCORES):
    """results: list of per-core {'out': [128, G]} -> full [B, 1] f32."""
    parts = []
    for c in range(ncores):
        o = np.asarray(results[c]["out"])  # [P, G]; out[p, g] = row g*128+p
        parts.append(o.T.reshape(-1))
    return np.concatenate(parts).reshape(-1, 1).astype(np.float32)


_CACHED_NC = None

FULL = dict(G=16, L=200, D=100, RPP=100, CPI=25)


def _quad_ok(idx):
    """quad_split needs >= 2*CPI quad-local tokens in EVERY row."""
    BPC = FULL["G"] * P
    qvpc = 4 * P * FULL["RPP"]
    for c in range(NCORES):
        rows = idx[c * BPC : (c + 1) * BPC]
        if ((rows // qvpc) == (c // 4)).sum(axis=1).min() < 2 * FULL["CPI"]:
            return False
    return True


def _get_nc(quad_split):
    global _CACHED_NC
    if _CACHED_NC is None or _CACHED_NC[1] != quad_split:
        _CACHED_NC = (build_program(**FULL, quad_split=quad_split), quad_split)
    return _CACHED_NC[0]


def run(word_idxs, embed_table, weights, trace=False, **spmd_kwargs):
    """Build (cached), run on the 8 cores, return (full_out, BassKernelResults)."""
    idx = np.asarray(word_idxs).astype(np.int32)
    # quad_split is dead on this stack: a 4-core-group AllGather cannot
    # use the Shared-output fast path ("needs >4 cores") and the local-out
    # bounce path is ~230us to data-ready (measured 1175us total); an
    # 8-core prefix collective would pay the same ~40-88us latency as the
    # full one (latency, not payload, dominates). Keep False.
    quad_split = False and _quad_ok(idx)
    nc = _get_nc(quad_split)
    in_maps = make_in_maps(
        idx,
        embed_table,
        weights,
        FULL["G"],
        FULL["L"],
        FULL["D"],
        FULL["RPP"],
        FULL["CPI"],
        quad_split=quad_split,
    )
    res = run_bass_kernel_spmd(
        nc, in_maps, core_ids=list(range(NCORES)), trace=trace, **spmd_kwargs
    )
    out = unshard_out(res.results, FULL["G"])
    return out, res


def kernel(word_idxs, embed_table, weights):
    out, _ = run(word_idxs, embed_table, weights, trace=False)
    return out

